# revision 44
# baseline (speedup 1.0000x reference)
"""CondMlp Trainium2 kernel.

Math (reference):
    xp = x @ W_pre + b_pre                 # [B, NI, DH]
    c  = query @ W_emb + b_emb             # [B, NO, DH]
    A  = xp @ W1[:DH] + b1                 # [B, NI, DH]   (host precompute, tiny)
    C2 = c @ W1[DH:]                       # [B, NO, DH]   (host precompute, tiny)
    h[b,i,o,:] = A[b,i,:] + C2[b,o,:]
    out[b,i,o,:] = gelu(h) @ W2 + b2       # [B, NI, NO, DOUT]

Sharding: 8 cores, core k handles batch b = k//2, NI-half h = k%2 (128 rows).

Design (constants measured from hardware traces/microbenchmarks):
  * The kernel is ACT+DVE-bound. Per core: gelu 59us (ACT-only, 1 elem/lane/
    cycle @1.2GHz), broadcast adds 50us (DVE tensor_scalar, 2x_1P cap with a
    tensor scalar operand, 196ns per [128,256]), PSUM drains 67us (1x
    port-bound fp32 reads; ACT 1.97us / DVE 2.29us per [128,2048]). Split
    across the two engines that's ~89us each; the matmuls (55us stream +
    ~25us LDWEIGHTS, partially hidden) keep PE at a similar level.
  * bf16 output stores (halves the 32MiB/core store traffic; ~0.2% rounding
    against a 2e-2 budget). Host untransposes + upcasts.
  * W2-stationary matmuls, N=512: out.T tiles = W2q.T @ g, so the gelu
    output feeds matmuls directly in its produced layout.
  * Drains alternate ACT/DVE per subgroup; stores are 1MiB (2 subgroups).
  * PE warmup matmuls flip the HAM clock-gate (1.2->2.4GHz) during the ramp;
    a scratch gelu pays the ~2.7us ACT table load early.
  * GPSIMD stock tensor_scalar measured 3.9us per [128,256] add (20x DVE):
    useless for compute; its ring only carries w2 loads + memsets.
"""

import numpy as np
import ml_dtypes

import concourse.bass as bass
import concourse.bacc as bacc
import concourse.mybir as mybir
from concourse.tile import TileContext
from concourse.bass_utils import run_bass_kernel_spmd

B, NI, NO = 4, 256, 256
DIN, DQ, DH, DOUT = 256, 256, 256, 256
NCORES = 8
RPC = (B * NI) // NCORES    # rows per core = 128
GROUP_ROWS = [16] * 8       # 16-row add/gelu groups
NSUB = RPC // 4             # 32 matmul subgroups of 4 rows
F32 = mybir.dt.float32
BF16 = mybir.dt.bfloat16

_nc_cache = None


def build_nc():
    nc = bacc.Bacc()

    c_t = nc.declare_dram_parameter("c_t", [DH, NO], BF16, isOutput=False)
    a_t = nc.declare_dram_parameter("a_t", [DH, RPC], F32, isOutput=False)
    # w2 quadrants [ch*2+d] = W2[ch*128:(ch+1)*128, d*128:(d+1)*128]
    w2 = nc.declare_dram_parameter("w2", [4, 128, 128], BF16, isOutput=False)
    # out[u, p, (s2, d, r, o)]: u = store unit (2 subgroups of 4 rows),
    # p = dout within chunk, free = s2*2048 + d*1024 + r*256 + o.
    out = nc.declare_dram_parameter("out", [NSUB // 2, 128, 4096], BF16,
                                    isOutput=True)

    gelu = mybir.ActivationFunctionType.Gelu

    with TileContext(nc) as tc:
        with (
            tc.tile_pool(name="const", bufs=1) as cpool,
            tc.tile_pool(name="h", bufs=2) as hpool,
            tc.tile_pool(name="g", bufs=2) as gpool,
            tc.tile_pool(name="ps", bufs=2, space="PSUM") as pspool,
            tc.tile_pool(name="ostage", bufs=4) as opool,
        ):
            ct, at, w2q = [], [], []
            for ch in range(2):
                t = cpool.tile([128, NO], BF16, tag=f"ct{ch}")
                ct.append(t)
                t = cpool.tile([128, RPC], F32, tag=f"at{ch}")
                at.append(t)
            for q in range(4):
                t = cpool.tile([128, 128], BF16, tag=f"w2q{q}")
                w2q.append(t)
            # Parallel loads across all three DMA-capable rings (sync/HWDGE,
            # scalar/HWDGE, gpsimd/SWDGE): ct0+at0 gate the first adds.
            nc.sync.dma_start(out=ct[0][:, :], in_=c_t[0:128, :])
            nc.scalar.dma_start(out=at[0][:, :], in_=a_t[0:128, :])
            nc.gpsimd.dma_start(out=ct[1][:, :], in_=c_t[128:256, :])
            nc.sync.dma_start(out=at[1][:, :], in_=a_t[128:256, :])
            for q in range(4):
                eng = nc.gpsimd if q % 2 else nc.sync
                eng.dma_start(out=w2q[q][:, :], in_=w2[q])

            # ACT warmup: pays the ~2.7us gelu table load during the ramp.
            scratch = cpool.tile([128, 2], F32, tag="scratch")
            nc.gpsimd.memset(scratch[:, :], 0.0)
            nc.scalar.activation(scratch[:, :], scratch[:, :], gelu)

            # PE warmup: dummy matmuls flip the HAM clock-gate to 8/8
            # (2.4 GHz) before the first real matmul.
            dummy = cpool.tile([128, 128], BF16, tag="dummy")
            nc.gpsimd.memset(dummy[:, :], 0.0)
            ps_w = pspool.tile([128, 2048], F32, tag="ps")
            for i in range(8):
                nc.tensor.matmul(out=ps_w[:, 0:128], lhsT=dummy[:, :],
                                 rhs=dummy[:, :], start=True, stop=True)
            # Chained to the ct1 load (~9.5us): keeps PE busy through the
            # ramp so the HAM MID-window never re-throttles to 1.2 GHz.
            for i in range(10):
                nc.tensor.matmul(out=ps_w[:, 0:128], lhsT=ct[1][:, 0:128],
                                 rhs=ct[1][:, 128:256], start=True, stop=True)

            drain_idx = 0
            row0 = 0
            for g, nrows in enumerate(GROUP_ROWS):
                # h/g free layout: (ch, r, o) -> (ch*nrows + r)*256 + o
                h_buf = hpool.tile([128, nrows * 512], BF16, tag="h")
                g_buf = gpool.tile([128, nrows * 512], BF16, tag="g")

                # First/last groups: 8-row half-gelus so the first/last
                # subgroup matmuls start after only half the adds (shorter
                # pipeline head/tail); middle groups use full-ch gelus.
                halves = 2 if g in (0, len(GROUP_ROWS) - 1) else 1
                hr = nrows // halves
                for hf in range(halves):
                    for ch in range(2):
                        for r in range(hf * hr, (hf + 1) * hr):
                            row = row0 + r
                            s = (ch * nrows + r) * 256
                            nc.vector.tensor_scalar_add(
                                out=h_buf[:, s:s + 256],
                                in0=ct[ch][:, :],
                                scalar1=at[ch][:, row:row + 1],
                            )
                        lo = (ch * nrows + hf * hr) * 256
                        nc.scalar.activation(
                            g_buf[:, lo:lo + hr * 256],
                            h_buf[:, lo:lo + hr * 256], gelu)

                if g == 0:
                    # Bridge the last PE-idle stretch before the first real
                    # matmuls (chained to the first gelu output).
                    for i in range(4):
                        nc.tensor.matmul(out=ps_w[:, 0:128],
                                         lhsT=g_buf[:, 0:128],
                                         rhs=g_buf[:, 0:128],
                                         start=True, stop=True)

                # 4-row matmul subgroups; 2 subgroups share one 1 MiB store.
                for s2 in range(nrows // 8):
                    ost = opool.tile([128, 4096], BF16, tag="ostage")
                    for s4i in range(2):
                        s4 = s2 * 2 + s4i
                        ps = pspool.tile([128, 2048], F32, tag="ps")
                        # ps free layout: (d, rpair, o) -> d*1024 + j*512 + o'
                        for d in range(2):
                            for ch in range(2):
                                for j in range(2):
                                    r0 = s4 * 4 + 2 * j
                                    nc.tensor.matmul(
                                        out=ps[:, d * 1024 + j * 512:
                                               d * 1024 + (j + 1) * 512],
                                        lhsT=w2q[ch * 2 + d][:, :],
                                        rhs=g_buf[:, (ch * nrows + r0) * 256:
                                                  (ch * nrows + r0) * 256 + 512],
                                        start=(ch == 0), stop=(ch == 1),
                                    )
                        dst = ost[:, s4i * 2048:(s4i + 1) * 2048]
                        # 15 ACT / 17 DVE overall: ACT (gelu-loaded) is the
                        # max engine. The last 6 drains are split across both
                        # engines (FD=1024 halves in parallel) so each PSUM
                        # tile frees in ~1.2us instead of ~2.3 during the
                        # drain-paced endgame; per-engine totals unchanged.
                        if drain_idx >= 26:
                            nc.scalar.copy(dst[:, 0:1024], ps[:, 0:1024])
                            nc.vector.tensor_copy(dst[:, 1024:2048],
                                                  ps[:, 1024:2048])
                        elif drain_idx % 2 == 1 and drain_idx != 17:
                            nc.scalar.copy(dst, ps[:, :])
                        else:
                            nc.vector.tensor_copy(dst, ps[:, :])
                        drain_idx += 1
                        u = row0 // 8 + s2
                        if u >= NSUB // 2 - 2:
                            # Tail units: store each half right after its
                            # drain so the final DMA flush is short.
                            nc.sync.dma_start(
                                out=out[u][:, s4i * 2048:(s4i + 1) * 2048],
                                in_=dst)
                    if u < NSUB // 2 - 2:
                        nc.sync.dma_start(out=out[u], in_=ost[:, :])
                row0 += nrows

    nc.finalize()
    return nc


def _get_nc():
    global _nc_cache
    if _nc_cache is None:
        _nc_cache = build_nc()
    return _nc_cache


def make_in_maps(x, query, W_pre, b_pre, W_emb, b_emb, W1, b1, W2, b2):
    x = np.asarray(x, np.float32)
    query = np.asarray(query, np.float32)
    W_pre = np.asarray(W_pre, np.float32)
    b_pre = np.asarray(b_pre, np.float32)
    W_emb = np.asarray(W_emb, np.float32)
    b_emb = np.asarray(b_emb, np.float32)
    W1 = np.asarray(W1, np.float32)
    b1 = np.asarray(b1, np.float32)
    W2 = np.asarray(W2, np.float32)

    xp = x.reshape(B * NI, DIN) @ W_pre + b_pre
    A = xp @ W1[:DH] + b1                       # [B*NI, DH]
    c = query.reshape(B * NO, DQ) @ W_emb + b_emb
    C2 = c @ W1[DH:]                            # [B*NO, DH]
    A = A.reshape(B, NI, DH)
    C2 = C2.reshape(B, NO, DH)

    # w2 quadrants [ch*2+d] = W2[ch*128:(ch+1)*128, d*128:(d+1)*128]
    w2b = np.ascontiguousarray(
        W2.reshape(2, 128, 2, 128).transpose(0, 2, 1, 3).reshape(4, 128, 128)
        .astype(ml_dtypes.bfloat16))
    in_maps = []
    for k in range(NCORES):
        b = k // 2
        hh = k % 2
        in_maps.append({
            "c_t": np.ascontiguousarray(C2[b].T.astype(ml_dtypes.bfloat16)),
            "a_t": np.ascontiguousarray(A[b, hh * 128:(hh + 1) * 128, :].T),
            "w2": w2b,
        })
    return in_maps


def run_on_device(in_maps, trace=False):
    nc = _get_nc()
    return run_bass_kernel_spmd(nc, in_maps, core_ids=list(range(NCORES)), trace=trace)


def assemble(results, b2):
    out = np.empty((B, NI, NO, DOUT), np.float32)
    for k in range(NCORES):
        b = k // 2
        hh = k % 2
        # dev out: [u, p, (s2, d, r, o)] -> out[b, (u*2+s2)*4+r, o, d*128+p]
        dev = results[k]["out"].reshape(NSUB // 2, 128, 2, 2, 4, 256)
        # axes (u, p, s2, d, r, o) -> (u, s2, r, o, d, p)
        dev = dev.transpose(0, 2, 4, 5, 3, 1).reshape(RPC, NO, DOUT)
        out[b, hh * 128:(hh + 1) * 128] = dev.astype(np.float32)
    b2 = np.asarray(b2, np.float32)
    if np.any(b2):
        out += b2
    return out


def kernel(x, query, W_pre, b_pre, W_emb, b_emb, W1, b1, W2, b2):
    in_maps = make_in_maps(x, query, W_pre, b_pre, W_emb, b_emb, W1, b1, W2, b2)
    res = run_on_device(in_maps, trace=False)
    return assemble(res.results, b2)


# revision 45
# speedup vs baseline: 1.0000x; 1.0000x over previous
"""CondMlp Trainium2 kernel.

Math (reference):
    xp = x @ W_pre + b_pre                 # [B, NI, DH]
    c  = query @ W_emb + b_emb             # [B, NO, DH]
    A  = xp @ W1[:DH] + b1                 # [B, NI, DH]   (host precompute, tiny)
    C2 = c @ W1[DH:]                       # [B, NO, DH]   (host precompute, tiny)
    h[b,i,o,:] = A[b,i,:] + C2[b,o,:]
    out[b,i,o,:] = gelu(h) @ W2 + b2       # [B, NI, NO, DOUT]

Sharding: 8 cores, core k handles batch b = k//2, NI-half h = k%2 (128 rows).

Design (constants measured from hardware traces/microbenchmarks):
  * The kernel is ACT+DVE-bound. Per core: gelu 59us (ACT-only, 1 elem/lane/
    cycle @1.2GHz), broadcast adds 50us (DVE tensor_scalar, 2x_1P cap with a
    tensor scalar operand, 196ns per [128,256]), PSUM drains 67us (1x
    port-bound fp32 reads; ACT 1.97us / DVE 2.29us per [128,2048]). Split
    across the two engines that's ~89us each; the matmuls (55us stream +
    ~25us LDWEIGHTS, partially hidden) keep PE at a similar level.
  * bf16 output stores (halves the 32MiB/core store traffic; ~0.2% rounding
    against a 2e-2 budget). Host untransposes + upcasts.
  * W2-stationary matmuls, N=512: out.T tiles = W2q.T @ g, so the gelu
    output feeds matmuls directly in its produced layout.
  * Drains alternate ACT/DVE per subgroup; stores are 1MiB (2 subgroups).
  * PE warmup matmuls flip the HAM clock-gate (1.2->2.4GHz) during the ramp;
    a scratch gelu pays the ~2.7us ACT table load early.
  * GPSIMD stock tensor_scalar measured 3.9us per [128,256] add (20x DVE):
    useless for compute; its ring only carries w2 loads + memsets.
"""

import numpy as np
import ml_dtypes

import concourse.bass as bass
import concourse.bacc as bacc
import concourse.mybir as mybir
from concourse.tile import TileContext
from concourse.bass_utils import run_bass_kernel_spmd

B, NI, NO = 4, 256, 256
DIN, DQ, DH, DOUT = 256, 256, 256, 256
NCORES = 8
RPC = (B * NI) // NCORES    # rows per core = 128
GROUP_ROWS = [16] * 8       # 16-row add/gelu groups
NSUB = RPC // 4             # 32 matmul subgroups of 4 rows
F32 = mybir.dt.float32
BF16 = mybir.dt.bfloat16

_nc_cache = None


def build_nc():
    nc = bacc.Bacc()

    c_t = nc.declare_dram_parameter("c_t", [DH, NO], BF16, isOutput=False)
    a_t = nc.declare_dram_parameter("a_t", [DH, RPC], F32, isOutput=False)
    # w2 quadrants [ch*2+d] = W2[ch*128:(ch+1)*128, d*128:(d+1)*128]
    w2 = nc.declare_dram_parameter("w2", [4, 128, 128], BF16, isOutput=False)
    # out[u, p, (s2, d, r, o)]: u = store unit (2 subgroups of 4 rows),
    # p = dout within chunk, free = s2*2048 + d*1024 + r*256 + o.
    out = nc.declare_dram_parameter("out", [NSUB // 2, 128, 4096], BF16,
                                    isOutput=True)

    gelu = mybir.ActivationFunctionType.Gelu

    with TileContext(nc) as tc:
        with (
            tc.tile_pool(name="const", bufs=1) as cpool,
            tc.tile_pool(name="h", bufs=2) as hpool,
            tc.tile_pool(name="g", bufs=2) as gpool,
            tc.tile_pool(name="ps", bufs=2, space="PSUM") as pspool,
            tc.tile_pool(name="ostage", bufs=4) as opool,
        ):
            ct, at, w2q = [], [], []
            for ch in range(2):
                t = cpool.tile([128, NO], BF16, tag=f"ct{ch}")
                ct.append(t)
                t = cpool.tile([128, RPC], F32, tag=f"at{ch}")
                at.append(t)
            for q in range(4):
                t = cpool.tile([128, 128], BF16, tag=f"w2q{q}")
                w2q.append(t)
            # Parallel loads across all three DMA-capable rings (sync/HWDGE,
            # scalar/HWDGE, gpsimd/SWDGE): ct0+at0 gate the first adds.
            nc.sync.dma_start(out=ct[0][:, :], in_=c_t[0:128, :])
            nc.scalar.dma_start(out=at[0][:, :], in_=a_t[0:128, :])
            nc.gpsimd.dma_start(out=ct[1][:, :], in_=c_t[128:256, :])
            nc.sync.dma_start(out=at[1][:, :], in_=a_t[128:256, :])
            for q in range(4):
                eng = nc.gpsimd if q % 2 else nc.sync
                eng.dma_start(out=w2q[q][:, :], in_=w2[q])

            # ACT warmup: pays the ~2.7us gelu table load during the ramp.
            scratch = cpool.tile([128, 2], F32, tag="scratch")
            nc.gpsimd.memset(scratch[:, :], 0.0)
            nc.scalar.activation(scratch[:, :], scratch[:, :], gelu)

            # PE warmup: dummy matmuls flip the HAM clock-gate to 8/8
            # (2.4 GHz) before the first real matmul.
            dummy = cpool.tile([128, 128], BF16, tag="dummy")
            nc.gpsimd.memset(dummy[:, :], 0.0)
            ps_w = pspool.tile([128, 2048], F32, tag="ps")
            for i in range(8):
                nc.tensor.matmul(out=ps_w[:, 0:128], lhsT=dummy[:, :],
                                 rhs=dummy[:, :], start=True, stop=True)
            # Chained to the ct1 load (~9.5us): keeps PE busy through the
            # ramp so the HAM MID-window never re-throttles to 1.2 GHz.
            for i in range(10):
                nc.tensor.matmul(out=ps_w[:, 0:128], lhsT=ct[1][:, 0:128],
                                 rhs=ct[1][:, 128:256], start=True, stop=True)

            drain_idx = 0
            row0 = 0
            for g, nrows in enumerate(GROUP_ROWS):
                # h/g free layout: (ch, r, o) -> (ch*nrows + r)*256 + o
                h_buf = hpool.tile([128, nrows * 512], BF16, tag="h")
                g_buf = gpool.tile([128, nrows * 512], BF16, tag="g")

                # First/last groups: 8-row half-gelus so the first/last
                # subgroup matmuls start after only half the adds (shorter
                # pipeline head/tail); middle groups use full-ch gelus.
                halves = 2 if g in (0, len(GROUP_ROWS) - 1) else 1
                hr = nrows // halves
                for hf in range(halves):
                    for ch in range(2):
                        for r in range(hf * hr, (hf + 1) * hr):
                            row = row0 + r
                            s = (ch * nrows + r) * 256
                            nc.vector.tensor_scalar_add(
                                out=h_buf[:, s:s + 256],
                                in0=ct[ch][:, :],
                                scalar1=at[ch][:, row:row + 1],
                            )
                        lo = (ch * nrows + hf * hr) * 256
                        nc.scalar.activation(
                            g_buf[:, lo:lo + hr * 256],
                            h_buf[:, lo:lo + hr * 256], gelu)

                if g == 0:
                    # Bridge the last PE-idle stretch before the first real
                    # matmuls (chained to the first gelu output).
                    for i in range(4):
                        nc.tensor.matmul(out=ps_w[:, 0:128],
                                         lhsT=g_buf[:, 0:128],
                                         rhs=g_buf[:, 0:128],
                                         start=True, stop=True)

                # 4-row matmul subgroups; 2 subgroups share one 1 MiB store.
                for s2 in range(nrows // 8):
                    ost = opool.tile([128, 4096], BF16, tag="ostage")
                    for s4i in range(2):
                        s4 = s2 * 2 + s4i
                        ps = pspool.tile([128, 2048], F32, tag="ps")
                        # ps free layout: (d, rpair, o) -> d*1024 + j*512 + o'
                        for d in range(2):
                            for ch in range(2):
                                for j in range(2):
                                    r0 = s4 * 4 + 2 * j
                                    nc.tensor.matmul(
                                        out=ps[:, d * 1024 + j * 512:
                                               d * 1024 + (j + 1) * 512],
                                        lhsT=w2q[ch * 2 + d][:, :],
                                        rhs=g_buf[:, (ch * nrows + r0) * 256:
                                                  (ch * nrows + r0) * 256 + 512],
                                        start=(ch == 0), stop=(ch == 1),
                                    )
                        dst = ost[:, s4i * 2048:(s4i + 1) * 2048]
                        # 15 ACT / 17 DVE: ACT (gelu-loaded) is the max
                        # engine; drain 17 moves to DVE.
                        if drain_idx % 2 == 1 and drain_idx != 17:
                            nc.scalar.copy(dst, ps[:, :])
                        else:
                            nc.vector.tensor_copy(dst, ps[:, :])
                        drain_idx += 1
                        u = row0 // 8 + s2
                        if u >= NSUB // 2 - 2:
                            # Tail units: store each half right after its
                            # drain so the final DMA flush is short.
                            nc.sync.dma_start(
                                out=out[u][:, s4i * 2048:(s4i + 1) * 2048],
                                in_=dst)
                    if u < NSUB // 2 - 2:
                        nc.sync.dma_start(out=out[u], in_=ost[:, :])
                row0 += nrows

    nc.finalize()
    return nc


def _get_nc():
    global _nc_cache
    if _nc_cache is None:
        _nc_cache = build_nc()
    return _nc_cache


def make_in_maps(x, query, W_pre, b_pre, W_emb, b_emb, W1, b1, W2, b2):
    x = np.asarray(x, np.float32)
    query = np.asarray(query, np.float32)
    W_pre = np.asarray(W_pre, np.float32)
    b_pre = np.asarray(b_pre, np.float32)
    W_emb = np.asarray(W_emb, np.float32)
    b_emb = np.asarray(b_emb, np.float32)
    W1 = np.asarray(W1, np.float32)
    b1 = np.asarray(b1, np.float32)
    W2 = np.asarray(W2, np.float32)

    xp = x.reshape(B * NI, DIN) @ W_pre + b_pre
    A = xp @ W1[:DH] + b1                       # [B*NI, DH]
    c = query.reshape(B * NO, DQ) @ W_emb + b_emb
    C2 = c @ W1[DH:]                            # [B*NO, DH]
    A = A.reshape(B, NI, DH)
    C2 = C2.reshape(B, NO, DH)

    # w2 quadrants [ch*2+d] = W2[ch*128:(ch+1)*128, d*128:(d+1)*128]
    w2b = np.ascontiguousarray(
        W2.reshape(2, 128, 2, 128).transpose(0, 2, 1, 3).reshape(4, 128, 128)
        .astype(ml_dtypes.bfloat16))
    in_maps = []
    for k in range(NCORES):
        b = k // 2
        hh = k % 2
        in_maps.append({
            "c_t": np.ascontiguousarray(C2[b].T.astype(ml_dtypes.bfloat16)),
            "a_t": np.ascontiguousarray(A[b, hh * 128:(hh + 1) * 128, :].T),
            "w2": w2b,
        })
    return in_maps


def run_on_device(in_maps, trace=False):
    nc = _get_nc()
    return run_bass_kernel_spmd(nc, in_maps, core_ids=list(range(NCORES)), trace=trace)


def assemble(results, b2):
    out = np.empty((B, NI, NO, DOUT), np.float32)
    for k in range(NCORES):
        b = k // 2
        hh = k % 2
        # dev out: [u, p, (s2, d, r, o)] -> out[b, (u*2+s2)*4+r, o, d*128+p]
        dev = results[k]["out"].reshape(NSUB // 2, 128, 2, 2, 4, 256)
        # axes (u, p, s2, d, r, o) -> (u, s2, r, o, d, p)
        dev = dev.transpose(0, 2, 4, 5, 3, 1).reshape(RPC, NO, DOUT)
        out[b, hh * 128:(hh + 1) * 128] = dev.astype(np.float32)
    b2 = np.asarray(b2, np.float32)
    if np.any(b2):
        out += b2
    return out


def kernel(x, query, W_pre, b_pre, W_emb, b_emb, W1, b1, W2, b2):
    in_maps = make_in_maps(x, query, W_pre, b_pre, W_emb, b_emb, W1, b1, W2, b2)
    res = run_on_device(in_maps, trace=False)
    return assemble(res.results, b2)
